# revision 1
# baseline (speedup 1.0000x reference)
"""Trainium2 Bass kernel for nn_DistanceLoss (retrieval_knn).

Computes 5-way logits from per-tuple Euclidean distances between
frame-pair embeddings of queries and a support set.

Math restructuring vs the reference:
  - emb[n,(i,j)] = relu(A[n,i] + B[n,j] + b) with A = x@W1.T, B = x@W2.T
    (W = [W1 | W2]); frame-level matmuls are 7.5x fewer FLOPs than
    embedding each of the 120 tuples separately.
  - min_u dist^2 = -2 * max_u (q.s - s^2/2 - q^2/2); sqrt deferred until
    after all min reductions; norms folded into the Gram PSUM via one
    K=128-padded bf16 matmul (row0/row1 carry [ones; -q^2/2] and
    [-s^2/2; ones]).

Embeddings are stored fp8e4m3 and the Gram runs as DoubleRow matmuls
(K=256 per instruction, 2 fp8 weights per PE cell); frame matmuls are
bf16; norms, sqrt and mean run in fp32-class precision.

Sharding: queries split across 8 cores (32 each); support set, W, b and
the class mask replicated.  No collectives; host concatenates logits.
"""

import sys
from contextlib import ExitStack

for _p in ("/opt/trn_rl_repo", "/root/.axon_site/_ro/trn_rl_repo"):
    if _p not in sys.path:
        sys.path.append(_p)

import ml_dtypes
import numpy as np

from concourse import bacc, mybir, tile
from concourse.bass import broadcast_tensor_aps
from concourse.bass_utils import run_bass_kernel_spmd

F32 = mybir.dt.float32
BF16 = mybir.dt.bfloat16
FP8 = mybir.dt.float8e4
DR = mybir.MatmulPerfMode.DoubleRow
RELU = mybir.ActivationFunctionType.Relu
COPY = mybir.ActivationFunctionType.Copy
SQRT = mybir.ActivationFunctionType.Sqrt
MAX = mybir.AluOpType.max
AXX = mybir.AxisListType.X

N_CORES = 8
NQ_TOT = 256
NQC = NQ_TOT // N_CORES    # queries per core
NS = 25                    # support samples
SEQ = 16
D = 2048                   # input dim per frame
H = 1024                   # embedding dim
T = 120                    # C(16,2) frame pairs
WAY = 5
KC = D // 128              # 16 contraction chunks per W half
MC = H // 128              # 8 h-chunks
QG = 8                     # queries per group
NGROUP = NQC // QG
NEG_BIG = -3.0e38          # empty-class sentinel; -2*NEG_BIG overflows to inf

# tuple (i,j), i<j, lexicographic; OFF[i] = first tuple index with first=i
OFF = [0]
for _i in range(15):
    OFF.append(OFF[-1] + (15 - _i))


def build_program():
    nc = bacc.Bacc("TRN2", target_bir_lowering=False, debug=False,
                   num_devices=N_CORES)

    qf_d = nc.dram_tensor("qf", [D, NQC * SEQ], BF16,
                          kind="ExternalInput").ap()
    sf_d = nc.dram_tensor("sf", [D, NS * SEQ], BF16,
                          kind="ExternalInput").ap()
    # W tiles pre-chunked on host: [m, kgrp, ksub, half, 128(d), 128(h)]
    w_d = nc.dram_tensor("w", [MC, KC // 4, 4, 2, 128, 128], BF16,
                         kind="ExternalInput").ap()
    b_d = nc.dram_tensor("b", [128, MC], F32, kind="ExternalInput").ap()
    mask_d = nc.dram_tensor("mask", [128, WAY, NS], F32,
                            kind="ExternalInput").ap()
    out_d = nc.dram_tensor("out", [1, NQC * WAY], F32,
                           kind="ExternalOutput").ap()

    with tile.TileContext(nc) as tc, ExitStack() as top:
        cpool = top.enter_context(tc.tile_pool(name="const", bufs=1))
        abpool = top.enter_context(tc.tile_pool(name="ab", bufs=1))
        sepool = top.enter_context(tc.tile_pool(name="sepool", bufs=1))

        ones = cpool.tile([128, 128], BF16)
        nc.vector.memset(ones[:, :], 1.0)
        onesf = cpool.tile([128, 1], F32)
        nc.vector.memset(onesf[:, :], 1.0)
        bt = cpool.tile([128, MC], F32)
        nc.sync.dma_start(bt[:, :], b_d)
        mask = cpool.tile([128, WAY, NS], F32)
        nc.sync.dma_start(mask[:, :, :], mask_d)
        # norm-fold operands padded to K=128 so the extra matmul runs at
        # full LDW/stream rate: row0 = -s2/2 (ones for q2L), row1 = ones
        # (-q2/2 for q2L), rows 2..127 = 0.
        s2L = cpool.tile([128, NS, T], BF16)
        nc.vector.memset(s2L[:, :, :], 0.0)
        nc.vector.memset(s2L[0:2, :, :], 1.0)

        qA = abpool.tile([128, MC, NQC, SEQ], BF16)
        qB = abpool.tile([128, MC, NQC, SEQ], BF16)
        se = sepool.tile([128, MC, NS, T], FP8)

        # ---- Phase M: frame matmuls + in-phase support expansion ----
        with (
            tc.tile_pool(name="frames", bufs=1) as fpool,
            tc.tile_pool(name="wtiles", bufs=6) as wpool,
            tc.tile_pool(name="sab", bufs=2) as sabpool,
            tc.tile_pool(name="pm", bufs=2, space="PSUM") as pm,
        ):
            qf = fpool.tile([128, KC, NQC * SEQ], BF16)
            nc.sync.dma_start(qf[:, :, :],
                              qf_d.rearrange("(k p) n -> p k n", p=128))
            sf = fpool.tile([128, KC, NS * SEQ], BF16)
            nc.sync.dma_start(sf[:, :, :],
                              sf_d.rearrange("(k p) n -> p k n", p=128))

            for m in range(MC):
                pAq = pm.tile([128, NQC, SEQ], F32, tag="pAq")
                pBq = pm.tile([128, NQC, SEQ], F32, tag="pBq")
                pAs = pm.tile([128, NS, SEQ], F32, tag="pAs")
                pBs = pm.tile([128, NS, SEQ], F32, tag="pBs")
                for kg in range(KC // 4):
                    w12 = wpool.tile([128, 4, 2, 128], BF16, tag="w12")
                    nc.sync.dma_start(
                        w12[:, :, :, :],
                        w_d[m, kg].rearrange("ks two p c -> p ks two c"))
                    for ks in range(4):
                        k = kg * 4 + ks
                        st, sp = k == 0, k == KC - 1
                        nc.tensor.matmul(pAq[:, :, :], w12[:, ks, 0],
                                         qf[:, k, :], start=st, stop=sp)
                        nc.tensor.matmul(pBq[:, :, :], w12[:, ks, 1],
                                         qf[:, k, :], start=st, stop=sp)
                        nc.tensor.matmul(pAs[:, :, :], w12[:, ks, 0],
                                         sf[:, k, :], start=st, stop=sp)
                        nc.tensor.matmul(pBs[:, :, :], w12[:, ks, 1],
                                         sf[:, k, :], start=st, stop=sp)
                nc.scalar.copy(qA[:, m], pAq[:, :, :])
                nc.scalar.copy(qB[:, m], pBq[:, :, :])
                sAm = sabpool.tile([128, NS, SEQ], BF16, tag="sA")
                nc.scalar.copy(sAm[:, :, :], pAs[:, :, :])
                sBm = sabpool.tile([128, NS, SEQ], BF16, tag="sB")
                nc.scalar.copy(sBm[:, :, :], pBs[:, :, :])
                # expand this chunk's support tuples + relu, in-phase
                for i in range(15):
                    c = 15 - i
                    a_ap, b_ap = broadcast_tensor_aps(
                        sAm[:, :, i:i + 1], sBm[:, :, i + 1:SEQ])
                    nc.gpsimd.tensor_add(se[:, m, :, OFF[i]:OFF[i] + c],
                                         a_ap, b_ap)
                nc.scalar.activation(se[:, m], se[:, m], RELU,
                                     bias=bt[:, m:m + 1], scale=1.0)

        # ---- Phase S2: -s2/2 into s2L row 0 ----
        with (
            tc.tile_pool(name="sq", bufs=4) as sqpool,
            tc.tile_pool(name="ps2", bufs=1, space="PSUM") as ps2,
        ):
            s2ps = []
            for blk in range(7):
                s2ps.append(ps2.tile([1, 4, T], F32, tag=f"ps2{blk}",
                                     name=f"s2ps{blk}"))
            for m in range(MC):
                for blk in range(7):
                    s0 = blk * 4
                    ns = min(4, NS - s0)
                    sq = sqpool.tile([128, 4, T], BF16, tag="sq")
                    nc.vector.tensor_mul(sq[:, :ns, :], se[:, m, s0:s0 + ns],
                                         se[:, m, s0:s0 + ns])
                    nc.tensor.matmul(s2ps[blk][:, :ns, :], ones[:, 0:1],
                                     sq[:, :ns, :],
                                     start=(m == 0), stop=(m == MC - 1))
            for blk in range(7):
                s0 = blk * 4
                ns = min(4, NS - s0)
                nc.scalar.activation(s2L[0:1, s0:s0 + ns],
                                     s2ps[blk][:, :ns, :], COPY, scale=-0.5)

        # ---- query groups ----
        with (
            tc.tile_pool(name="qe", bufs=2) as qepool,
            tc.tile_pool(name="qtmp", bufs=2) as qtmp,
            tc.tile_pool(name="qsq", bufs=4) as qsqpool,
            tc.tile_pool(name="small", bufs=2) as spool,
            tc.tile_pool(name="pq2", bufs=2, space="PSUM") as pq2,
            tc.tile_pool(name="pd", bufs=4, space="PSUM") as pdp,
            tc.tile_pool(name="plog", bufs=1, space="PSUM") as plp,
        ):
            plog = plp.tile([1, NQC * WAY], F32)
            for g in range(NGROUP):
                q0 = g * QG
                qe = qepool.tile([128, MC, QG, 128], FP8, tag="qe")
                nc.vector.memset(qe[:, :, :, T:128], 0.0)
                for i in range(15):
                    c = 15 - i
                    a_ap, b_ap = broadcast_tensor_aps(
                        qA[:, :, q0:q0 + QG, i:i + 1],
                        qB[:, :, q0:q0 + QG, i + 1:SEQ])
                    nc.vector.tensor_add(qe[:, :, :, OFF[i]:OFF[i] + c],
                                         a_ap, b_ap)
                for m in range(MC):
                    nc.scalar.activation(qe[:, m], qe[:, m], RELU,
                                         bias=bt[:, m:m + 1], scale=1.0)

                # -q2/2 for this group -> q2L row 1 (via DMA partition hop)
                q2n = qtmp.tile([1, QG, 128], BF16, tag="q2n")
                for blk in range(2):
                    c0 = blk * 4
                    p2 = pq2.tile([1, 4, 128], F32, tag="pq2")
                    for m in range(MC):
                        sq = qsqpool.tile([128, 4, 128], BF16, tag="qsq")
                        nc.vector.tensor_mul(sq[:, :, :],
                                             qe[:, m, c0:c0 + 4],
                                             qe[:, m, c0:c0 + 4])
                        nc.tensor.matmul(p2[:, :, :], ones[:, 0:1],
                                         sq[:, :, :],
                                         start=(m == 0), stop=(m == MC - 1))
                    nc.scalar.activation(q2n[0:1, c0:c0 + 4], p2[:, :, :],
                                         COPY, scale=-0.5)
                q2L = qtmp.tile([128, QG, 128], BF16, tag="q2L")
                nc.vector.memset(q2L[:, :, :], 0.0)
                nc.vector.memset(q2L[0:2, :, :], 1.0)
                nc.sync.dma_start(q2L[1:2, :, :], q2n[0:1, :, :])

                for q in range(QG):
                    maxm = spool.tile([128, 1, NS], F32, tag="maxm")
                    for blk in range(7):
                        s0 = blk * 4
                        ns = min(4, NS - s0)
                        pdt = pdp.tile([128, 4, T], F32, tag="pd")
                        for j in range(MC // 2):
                            nc.tensor.matmul(pdt[:, :ns, :],
                                             qe[:, 2 * j:2 * j + 2, q],
                                             se[:, 2 * j:2 * j + 2,
                                                s0:s0 + ns],
                                             start=(j == 0), stop=False,
                                             perf_mode=DR)
                        nc.tensor.matmul(pdt[:, :ns, :], q2L[:, q],
                                         s2L[:, s0:s0 + ns],
                                         start=False, stop=True)
                        nc.vector.tensor_reduce(maxm[:, 0, s0:s0 + ns],
                                                pdt[:, :ns, :],
                                                axis=AXX, op=MAX)
                    masked = spool.tile([128, WAY, NS], F32, tag="masked")
                    mm_ap, mk_ap = broadcast_tensor_aps(maxm[:, 0:1, :],
                                                        mask[:, :, :])
                    nc.vector.tensor_add(masked[:, :, :], mm_ap, mk_ap)
                    mc_t = spool.tile([128, WAY], F32, tag="mc")
                    nc.vector.tensor_reduce(mc_t[:, :], masked[:, :, :],
                                            axis=AXX, op=MAX)
                    dt_ = spool.tile([128, WAY], F32, tag="d")
                    nc.vector.tensor_scalar(dt_[:, :], mc_t[:, :],
                                            -2.0, 1e-12,
                                            mybir.AluOpType.mult, MAX)
                    nc.scalar.activation(dt_[:, :], dt_[:, :], SQRT)
                    qi = q0 + q
                    nc.tensor.matmul(plog[0:1, qi * WAY:(qi + 1) * WAY],
                                     onesf[0:T, :], dt_[0:T, :],
                                     start=True, stop=True)

            louts = cpool.tile([1, NQC * WAY], F32)
            nc.scalar.activation(louts[:, :], plog[:, :], COPY,
                                 scale=-1.0 / T)
            nc.sync.dma_start(out_d, louts[:, :])
    nc.compile()
    return nc


_NC_CACHE = None
LAST = None


def kernel(support_set, queries, support_labels, W, b):
    global _NC_CACHE, LAST
    support_set = np.asarray(support_set, dtype=np.float32)
    queries = np.asarray(queries, dtype=np.float32)
    support_labels = np.asarray(support_labels)
    W = np.asarray(W, dtype=np.float32)
    b = np.asarray(b, dtype=np.float32)
    bf = ml_dtypes.bfloat16

    # host-side layout prep (pure data movement + bf16 cast)
    sf = np.ascontiguousarray(support_set.reshape(NS * SEQ, D).T.astype(bf))
    wt = np.ascontiguousarray(
        W.reshape(MC, 128, 2, KC // 4, 4, 128)
        .transpose(0, 3, 4, 2, 5, 1).astype(bf))
    bt = np.ascontiguousarray(b.reshape(MC, 128).T)
    maskv = np.where(support_labels[None, :] == np.arange(WAY)[:, None],
                     np.float32(0.0), np.float32(NEG_BIG)).astype(np.float32)
    maskrep = np.ascontiguousarray(
        np.broadcast_to(maskv[None], (128, WAY, NS)))

    in_maps = []
    for c in range(N_CORES):
        qfc = np.ascontiguousarray(
            queries[c * NQC:(c + 1) * NQC].reshape(NQC * SEQ, D).T.astype(bf))
        in_maps.append({"qf": qfc, "sf": sf, "w": wt, "b": bt,
                        "mask": maskrep})

    if _NC_CACHE is None:
        _NC_CACHE = build_program()
    res = run_bass_kernel_spmd(_NC_CACHE, in_maps, list(range(N_CORES)))
    LAST = res
    outs = [res.results[c]["out"].reshape(NQC, WAY) for c in range(N_CORES)]
    return np.concatenate(outs, axis=0)


if __name__ == "__main__":
    rng = np.random.default_rng(0)
    out = kernel(
        rng.standard_normal((NS, SEQ, D)).astype(np.float32),
        rng.standard_normal((NQ_TOT, SEQ, D)).astype(np.float32),
        (np.arange(NS) % WAY).astype(np.int32),
        (rng.standard_normal((H, 2 * D)) / np.sqrt(2 * D)).astype(np.float32),
        (rng.standard_normal(H) * 0.01).astype(np.float32),
    )
    print(out.shape, out[:2])



# revision 11
# speedup vs baseline: 1.0078x; 1.0078x over previous
"""Trainium2 Bass kernel for nn_DistanceLoss (retrieval_knn).

Computes 5-way logits from per-tuple Euclidean distances between
frame-pair embeddings of queries and a support set.

Math restructuring vs the reference:
  - emb[n,(i,j)] = relu(A[n,i] + B[n,j] + b) with A = x@W1.T, B = x@W2.T
    (W = [W1 | W2]); frame-level matmuls are 7.5x fewer FLOPs than
    embedding each of the 120 tuples separately.
  - Tuples are enumerated gap-major ((i, i+g) for g=1..15); min and mean
    over tuples are order-invariant, and this order turns the pair
    expansion into contiguous vector adds with no broadcast APs.
  - min_u dist^2 = q^2 + min_u (s^2 - 2 q.s); support samples are
    reordered class-major on the host so each class is a contiguous
    column block.  -s^2/2 is folded into the Gram PSUM by a K=1 ones-row
    matmul appended to each accumulation chain; the class min is then a
    plain MAX reduction read straight out of PSUM, and q^2 and the -2
    scale fold into the final sqrt activation: dist = sqrt(-2*max + q^2).

All heavy matmuls run fp8e4m3 DoubleRow (K=256 per instruction); W and
b are pre-scaled by 32 on the host so W fits fp8 dynamic range, and the
1/32 is folded into the final mean scale.  Query tuple columns are
tuple-major and packed densely into 30 stationary tiles of 128.

Sharding: queries split across 8 cores (32 each); support set, W, b
replicated.  No collectives; host concatenates logits.
"""

import sys
from contextlib import ExitStack

for _p in ("/opt/trn_rl_repo", "/root/.axon_site/_ro/trn_rl_repo"):
    if _p not in sys.path:
        sys.path.append(_p)

import ml_dtypes
import numpy as np

from concourse import bacc, mybir, tile
from concourse.bass_utils import run_bass_kernel_spmd

F32 = mybir.dt.float32
BF16 = mybir.dt.bfloat16
FP8 = mybir.dt.float8e4
DR = mybir.MatmulPerfMode.DoubleRow
RELU = mybir.ActivationFunctionType.Relu
COPY = mybir.ActivationFunctionType.Copy
SQRT = mybir.ActivationFunctionType.Sqrt
SQUARE = mybir.ActivationFunctionType.Square
MAX = mybir.AluOpType.max
ADD = mybir.AluOpType.add
AXX = mybir.AxisListType.X

N_CORES = 8
NQ_TOT = 256
NQC = NQ_TOT // N_CORES    # queries per core
NS = 25                    # support samples
SEQ = 16
D = 2048                   # input dim per frame
H = 1024                   # embedding dim
T = 120                    # C(16,2) frame pairs
WAY = 5
MC = H // 128              # 8 h-chunks
KC = D // 256              # 8 fp8-DR contraction chunks per W half
NT = NS * T                # 3000 support tuples
NQT = NQC * T              # 3840 query tuples
NTILE = NQT // 128         # 30 stationary query-tuple tiles
SCL = 32.0                 # host W/b scale (fp8 range); undone in final mean
BANK = 512                 # psum bank capacity in f32 columns

# gap-major tuple order: gap g=1..15, GOFF[g] = first tuple index of gap g
GOFF = [0, 0]
for _g in range(1, 15):
    GOFF.append(GOFF[-1] + (16 - _g))


def _bank_pieces(lo, hi):
    """Split [lo,hi) at PSUM bank boundaries (multiples of BANK)."""
    out = []
    while lo < hi:
        nxt = min(hi, (lo // BANK + 1) * BANK)
        out.append((lo, nxt))
        lo = nxt
    return out


def build_program(class_counts):
    """class_counts: support samples per class after class-major reorder."""
    bounds = [0]
    for c in class_counts:
        bounds.append(bounds[-1] + c * T)
    assert bounds[-1] == NT

    nc = bacc.Bacc("TRN2", target_bir_lowering=False, debug=False,
                   num_devices=N_CORES)

    qf_d = nc.dram_tensor("qf", [128, KC, 2, SEQ * NQC], FP8,
                          kind="ExternalInput").ap()
    sf_d = nc.dram_tensor("sf", [128, KC, 2, NS * SEQ], FP8,
                          kind="ExternalInput").ap()
    w_d = nc.dram_tensor("w", [128, MC, 2, KC, 2, 128], FP8,
                         kind="ExternalInput").ap()
    b_d = nc.dram_tensor("b", [128, MC], F32, kind="ExternalInput").ap()
    seg_d = nc.dram_tensor("seg", [128, NTILE, NQC], BF16,
                           kind="ExternalInput").ap()
    out_d = nc.dram_tensor("out", [NQC, WAY], F32,
                           kind="ExternalOutput").ap()

    with tile.TileContext(nc) as tc, ExitStack() as top:
        cpool = top.enter_context(tc.tile_pool(name="const", bufs=1))
        epool = top.enter_context(tc.tile_pool(name="emb", bufs=1))

        ones = cpool.tile([128, 128], BF16)
        nc.vector.memset(ones[:, :], 1.0)
        idf = cpool.tile([1, 1], F32)
        nc.vector.memset(idf[:, :], 1.0)
        bt = cpool.tile([128, MC], F32)
        nc.sync.dma_start(bt[:, :], b_d)
        segsb = cpool.tile([128, NTILE, NQC], BF16)
        nc.sync.dma_start(segsb[:, :, :], seg_d)

        qe = epool.tile([128, MC, NQT], FP8)       # query tuple embeddings
        se = epool.tile([128, MC, NT], FP8)        # support tuple embeddings
        s2neg = epool.tile([1, NT], BF16)          # -s^2/2 row
        q2sb = epool.tile([128, NTILE], F32)       # q^2 per packed tile row

        se4 = se[:, :, :].rearrange("p m (s t) -> p m s t", s=NS)

        # ---- Phase F: frame matmuls (fp8 DR) + per-m tuple expansion ----
        with (
            tc.tile_pool(name="frames", bufs=1) as fpool,
            tc.tile_pool(name="wtiles", bufs=1) as wpool,
            tc.tile_pool(name="fab", bufs=1) as abpool,
            tc.tile_pool(name="pre", bufs=2) as prepool,
            tc.tile_pool(name="pf", bufs=2, space="PSUM") as pf,
        ):
            qf = fpool.tile([128, KC, 2, SEQ * NQC], FP8)
            sf = fpool.tile([128, KC, 2, NS * SEQ], FP8)
            wt = wpool.tile([128, MC, 2, KC, 2, 128], FP8)
            for mg in range(4):
                nc.sync.dma_start(wt[:, 2 * mg:2 * mg + 2],
                                  w_d[:, 2 * mg:2 * mg + 2])
            nc.sync.dma_start(qf[:, :, :, :], qf_d)
            nc.sync.dma_start(sf[:, :, :, :], sf_d)

            # query frames frame-major: [128, MC, half, SEQ, NQC]
            qAB = abpool.tile([128, MC, 2, SEQ, NQC], BF16)
            # support frames sample-major: [128, MC, half, NS, SEQ]
            sAB = abpool.tile([128, MC, 2, NS, SEQ], BF16)

            for m in range(MC):
                for half in range(2):
                    pq = pf.tile([128, SEQ * NQC], F32, tag="pq")
                    ps = pf.tile([128, NS * SEQ], F32, tag="ps")
                    for k in range(KC):
                        st, sp = k == 0, k == KC - 1
                        nc.tensor.matmul(pq[:, :], wt[:, m, half, k],
                                         qf[:, k], start=st, stop=sp,
                                         perf_mode=DR)
                        nc.tensor.matmul(ps[:, :], wt[:, m, half, k],
                                         sf[:, k], start=st, stop=sp,
                                         perf_mode=DR)
                    nc.scalar.copy(qAB[:, m, half], pq[:, :])
                    nc.scalar.copy(sAB[:, m, half], ps[:, :])
                # gap-major expansion: tuples (i, i+g); all APs contiguous
                qpre = prepool.tile([128, NQT], BF16, tag="qpre")
                spre = prepool.tile([128, NS, T], BF16, tag="spre")
                for g in range(1, SEQ):
                    n = SEQ - g
                    nc.vector.tensor_tensor(
                        out=qpre[:, GOFF[g] * NQC:(GOFF[g] + n) * NQC],
                        in0=qAB[:, m, 0, 0:n],
                        in1=qAB[:, m, 1, g:SEQ], op=ADD)
                    nc.gpsimd.tensor_tensor(
                        out=spre[:, :, GOFF[g]:GOFF[g] + n],
                        in0=sAB[:, m, 0, :, 0:n],
                        in1=sAB[:, m, 1, :, g:SEQ], op=ADD)
                nc.scalar.activation(qe[:, m], qpre[:, :], RELU,
                                     bias=bt[:, m:m + 1], scale=1.0)
                nc.scalar.activation(se[:, m], spre[:, :, :], RELU,
                                     bias=bt[:, m:m + 1], scale=1.0)

        # ---- Phase N: -s^2/2 row, q^2 row -> per-tile-row column ----
        with (
            tc.tile_pool(name="sq", bufs=2) as sqpool,
            tc.tile_pool(name="q2row", bufs=1) as q2pool,
            tc.tile_pool(name="pn", bufs=2, space="PSUM") as pn,
            tc.tile_pool(name="pt", bufs=1, space="PSUM") as pt,
        ):
            q2row = q2pool.tile([1, NQT], F32)
            for which, tot, src in ((0, NT, se), (1, NQT, qe)):
                for c0 in range(0, tot, BANK):
                    n = min(BANK, tot - c0)
                    sq = sqpool.tile([128, MC, BANK], BF16, tag="sq")
                    pnrm = pn.tile([1, BANK], F32, tag="pn")
                    for m in range(MC):
                        nc.scalar.activation(sq[:, m, :n],
                                             src[:, m, c0:c0 + n], SQUARE)
                        nc.tensor.matmul(pnrm[:, :n], ones[:, 0:1],
                                         sq[:, m, :n],
                                         start=(m == 0), stop=(m == MC - 1))
                    if which == 0:
                        nc.scalar.activation(s2neg[0:1, c0:c0 + n],
                                             pnrm[:, :n], COPY, scale=-0.5)
                    else:
                        nc.scalar.copy(q2row[0:1, c0:c0 + n], pnrm[:, :n])
            # transpose q2row -> [128, NTILE] via PE ([1,128] -> [128,1])
            ptr = pt.tile([128, NTILE], F32)
            for t in range(NTILE):
                nc.tensor.matmul(ptr[:, t:t + 1],
                                 q2row[0:1, 128 * t:128 * (t + 1)],
                                 idf[0:1, 0:1], is_transpose=True)
            nc.scalar.copy(q2sb[:, :], ptr[:, :])

        # ---- Phase G: Gram (+norm fold) + class max + sqrt + mean ----
        chunks = _bank_pieces(0, NT)
        cls_pieces = [_bank_pieces(bounds[c], bounds[c + 1])
                      for c in range(WAY)]
        uniform2 = all(len(p) == 2 for p in cls_pieces)
        with (
            tc.tile_pool(name="gps", bufs=1, space="PSUM") as gp,
            tc.tile_pool(name="mps", bufs=1, space="PSUM") as mp,
            tc.tile_pool(name="dts", bufs=1) as dpool,
            tc.tile_pool(name="acc", bufs=2) as apool,
        ):
            gb = [gp.tile([128, c1 - c0], F32, name=f"gb{ci}")
                  for ci, (c0, c1) in enumerate(chunks)]
            mpsum = mp.tile([NQC, NTILE, WAY], F32)
            dtsb = dpool.tile([128, NTILE, WAY], BF16)

            def mean_matmul(t):
                nc.tensor.matmul(mpsum[:, t, :],
                                 segsb[:, t, :], dtsb[:, t, :],
                                 start=True, stop=True)

            for t in range(NTILE):
                for ci, (c0, c1) in enumerate(chunks):
                    for kc in range(MC // 2):
                        nc.tensor.matmul(
                            gb[ci][:, :],
                            qe[:, 2 * kc:2 * kc + 2, 128 * t:128 * (t + 1)],
                            se[:, 2 * kc:2 * kc + 2, c0:c1],
                            start=(kc == 0), stop=False,
                            perf_mode=DR)
                    # fold -s^2/2 into the chain (ones-row x s2neg row)
                    nc.tensor.matmul(gb[ci][:, :], ones[0:1, :],
                                     s2neg[0:1, c0:c1],
                                     start=False, stop=True)
                if t > 0:
                    mean_matmul(t - 1)  # deferred: avoids PE wait on DVE
                # class max of (g - s^2/2) straight from PSUM bank pieces
                mp2 = apool.tile([128, WAY, 2], F32, tag="mp2")
                maxacc = apool.tile([128, WAY], F32, tag="acc")
                for cls in range(WAY):
                    pieces = cls_pieces[cls]
                    if uniform2:
                        for pi, (p0, p1) in enumerate(pieces):
                            ci = p0 // BANK
                            b0 = p0 - chunks[ci][0]
                            nc.vector.tensor_reduce(
                                mp2[:, cls, pi:pi + 1],
                                gb[ci][:, b0:b0 + p1 - p0],
                                axis=AXX, op=MAX)
                    else:
                        if not pieces:
                            nc.vector.memset(maxacc[:, cls:cls + 1], -3.0e38)
                            continue
                        for pi, (p0, p1) in enumerate(pieces):
                            ci = p0 // BANK
                            b0 = p0 - chunks[ci][0]
                            dst = (maxacc[:, cls:cls + 1] if pi == 0
                                   else mp2[:, 0, 0:1])
                            nc.vector.tensor_reduce(
                                dst, gb[ci][:, b0:b0 + p1 - p0],
                                axis=AXX, op=MAX)
                            if pi > 0:
                                nc.vector.tensor_tensor(
                                    out=maxacc[:, cls:cls + 1],
                                    in0=maxacc[:, cls:cls + 1],
                                    in1=mp2[:, 0, 0:1], op=MAX)
                if uniform2:
                    nc.vector.tensor_reduce(maxacc[:, :], mp2[:, :, :],
                                            axis=AXX, op=MAX)
                # dist = sqrt(-2*max + q^2)
                nc.scalar.activation(dtsb[:, t, :], maxacc[:, :], SQRT,
                                     bias=q2sb[:, t:t + 1], scale=-2.0)
            mean_matmul(NTILE - 1)

            plog = apool.tile([NQC, WAY], F32, tag="plog")
            for cls in range(WAY):
                nc.vector.tensor_reduce(plog[:, cls:cls + 1],
                                        mpsum[:, :, cls], axis=AXX, op=ADD)
            louts = apool.tile([NQC, WAY], F32, tag="louts")
            nc.scalar.activation(louts[:, :], plog[:, :], COPY,
                                 scale=-1.0 / (T * SCL))
            nc.sync.dma_start(out_d, louts[:, :])
    nc.compile()
    return nc


_NC_CACHE = {}
LAST = None


def _frames_fp8(x, n, frame_major):
    """[n, SEQ, D] f32 -> [128, KC, 2, cols] fp8 (DR moving layout).

    frame_major: cols = frame*n + sample; else cols = sample*SEQ + frame.
    """
    f8 = ml_dtypes.float8_e4m3
    if frame_major:
        fr = x.transpose(1, 0, 2).reshape(SEQ * n, KC, 2, 128)
    else:
        fr = x.reshape(n * SEQ, KC, 2, 128)
    return np.ascontiguousarray(fr.transpose(3, 1, 2, 0).astype(f8))


def kernel(support_set, queries, support_labels, W, b):
    global LAST
    support_set = np.asarray(support_set, dtype=np.float32)
    queries = np.asarray(queries, dtype=np.float32)
    support_labels = np.asarray(support_labels)
    W = np.asarray(W, dtype=np.float32)
    b = np.asarray(b, dtype=np.float32)
    f8 = ml_dtypes.float8_e4m3

    # class-major support reorder (class blocks contiguous)
    perm = np.argsort(support_labels, kind="stable")
    counts = tuple(int((support_labels == c).sum()) for c in range(WAY))
    sf = _frames_fp8(support_set[perm], NS, frame_major=False)

    # W: [p, m, half, kc, pair, hcol], scaled into fp8 range
    wt = np.ascontiguousarray(
        (W * SCL).reshape(MC, 128, 2, KC, 2, 128)
        .transpose(5, 0, 2, 3, 4, 1).astype(f8))
    bt = np.ascontiguousarray((b * SCL).reshape(MC, 128).T)

    # segment matrix: query-tuple cols are tuple-major -> query = col % NQC
    seg = np.zeros((128, NTILE, NQC), dtype=np.float32)
    for t in range(NTILE):
        for r in range(128):
            seg[r, t, (128 * t + r) % NQC] = 1.0
    seg = seg.astype(ml_dtypes.bfloat16)

    in_maps = []
    for c in range(N_CORES):
        qfc = _frames_fp8(queries[c * NQC:(c + 1) * NQC], NQC,
                          frame_major=True)
        in_maps.append({"qf": qfc, "sf": sf, "w": wt, "b": bt, "seg": seg})

    if counts not in _NC_CACHE:
        _NC_CACHE[counts] = build_program(counts)
    res = run_bass_kernel_spmd(_NC_CACHE[counts], in_maps,
                               list(range(N_CORES)))
    LAST = res
    outs = [res.results[c]["out"] for c in range(N_CORES)]
    return np.concatenate(outs, axis=0)


if __name__ == "__main__":
    rng = np.random.default_rng(0)
    out = kernel(
        rng.standard_normal((NS, SEQ, D)).astype(np.float32),
        rng.standard_normal((NQ_TOT, SEQ, D)).astype(np.float32),
        (np.arange(NS) % WAY).astype(np.int32),
        (rng.standard_normal((H, 2 * D)) / np.sqrt(2 * D)).astype(np.float32),
        (rng.standard_normal(H) * 0.01).astype(np.float32),
    )
    print(out.shape, out[:2])
